# revision 6
# baseline (speedup 1.0000x reference)
"""Trainium2 Bass kernel for the MiniBatchAUC pairwise surrogate loss.

Math: with s = sigmoid(logits), pos/neg the 0/1 target masks,
    loss_sum = sum_{i in P, j in N} (1 - s_i + s_j)^2
factorizes exactly (expand the square; the double sum separates):
    loss_sum = n_neg * Sp2 + 2 * Sp1 * Sn1 + n_pos * Sn2
      Sp1 = sum_P (1-s),  Sp2 = sum_P (1-s)^2,
      Sn1 = sum_N s,      Sn2 = sum_N s^2,
and with c = sum T, m1 = sum T*s, m2 = sum T*s^2, g1 = sum s, g2 = sum s^2:
      Sp1 = c - m1, Sp2 = c - 2*m1 + m2, Sn1 = g1 - m1, Sn2 = g2 - m2.
So the O(N^2) pairwise matrix is never materialized: each core reduces its
2048-element shard to 5 per-partition partial sums; the host all-reduces
the per-core partials and applies the closed form.

Per-core device program (SPMD, identical on all 8 cores):
  - one DMA in: [128, 32] f32 tile = logits(16 cols) | targets(16)
  - ACT: s = sigmoid(L) (fused accum -> per-partition sum s),
         count = Copy(T) (fused accum -> per-partition sum T)
  - DVE: s*s, T*s, (T*s)*s multiplies + reduce_sum of each
    (tensor_tensor_reduce crashes this terminal's runtime; ACT Square in the
     s -> s2 chain is slower than overlapping the multiply on DVE)
  - one DMA out: the [128, 5] per-partition partials (2.5 KB)
No PE/PSUM involvement - the partition reduction is part of the host-side
all-reduce of partials (TimelineSim: 6794 ns vs 7537 ns with an
on-device ones-matmul partition reduction).

Written in raw bacc (manual semaphores, no TileContext) so the program
carries no Tile exit drain / EVSEM butterfly: 6589 ns modeled vs 6794 ns
for the identical Tile-scheduled program, and the real-hardware tail cost
of the Tile barrier is documented as multi-microsecond. Same-engine RAW
hazards are semaphore-chained (deep pipelines reorder retirement); the
schedule was validated race-free in CoreSim and bit-exact on hardware.
"""

import numpy as np

try:
    import concourse.bass as bass
except ImportError:  # concourse ships in the container, not on sys.path
    import sys

    sys.path.insert(0, "/opt/trn_rl_repo")
    import concourse.bass as bass

import concourse.tile as tile
from concourse import bacc, mybir
from concourse import bass_utils

N = 16384
NCORES = 8
SHARD = N // NCORES  # 2048 elements per core
P = 128  # SBUF partitions
F = SHARD // P  # 16 free elements per partition

f32 = mybir.dt.float32

_CACHE: dict = {}


def _build():
    nc = bacc.Bacc(
        "TRN2",
        target_bir_lowering=False,
        debug=False,
        enable_asserts=False,
        num_devices=NCORES,
    )
    x_dram = nc.dram_tensor("x", [P, 2 * F], f32, kind="ExternalInput").ap()
    o_dram = nc.dram_tensor("o", [P, 5], f32, kind="ExternalOutput").ap()

    Sig = mybir.ActivationFunctionType.Sigmoid
    Copy = mybir.ActivationFunctionType.Copy
    X = mybir.AxisListType.X

    # Raw bacc with manual semaphores: no TileContext, so the Tile exit
    # drain + EVSEM butterfly never enters the program.
    with (
        nc.sbuf_tensor([P, 2 * F], f32) as x,
        nc.sbuf_tensor([P, F], f32) as s,
        nc.sbuf_tensor([P, F], f32) as s2,
        nc.sbuf_tensor([P, F], f32) as tcnt,
        nc.sbuf_tensor([P, F], f32) as ts,
        nc.sbuf_tensor([P, F], f32) as ts2,
        nc.sbuf_tensor([P, 5], f32) as r,  # g1 | g2 | c | m1 | m2
        nc.semaphore() as dsem,
        nc.semaphore() as asem,
        nc.semaphore() as vsem,
        nc.semaphore() as osem,
        nc.Block() as block,
    ):
        L = x[:, 0:F]
        T = x[:, F : 2 * F]

        @block.sync
        def _(sync):
            sync.dma_start(x[:], x_dram).then_inc(dsem, 16)
            sync.wait_ge(asem, 2)  # both ACT accums landed in r
            sync.wait_ge(vsem, 6)  # all DVE muls + reduces landed in r
            sync.dma_start(o_dram, r[:]).then_inc(osem, 16)
            sync.wait_ge(osem, 16)  # out-DMA complete before program end

        @block.scalar
        def _(scalar):
            scalar.wait_ge(dsem, 16)
            nc.scalar.activation(s[:], L, Sig, accum_out=r[:, 0:1]).then_inc(asem, 1)
            nc.scalar.activation(tcnt[:], T, Copy, accum_out=r[:, 2:3]).then_inc(
                asem, 1
            )

        @block.vector
        def _(vector):
            # Deep engine pipelines: same-engine RAW hazards need sem chains
            # (the race detector rejects back-to-back dependent DVE ops).
            vector.wait_ge(dsem, 16)  # T in SBUF
            vector.wait_ge(asem, 1)  # s written
            nc.vector.tensor_mul(ts[:], T, s[:]).then_inc(vsem, 1)
            nc.vector.tensor_mul(s2[:], s[:], s[:]).then_inc(vsem, 1)
            vector.wait_ge(vsem, 1)  # ts retired
            nc.vector.tensor_mul(ts2[:], ts[:], s[:]).then_inc(vsem, 1)
            nc.vector.reduce_sum(r[:, 3:4], ts[:], axis=X).then_inc(vsem, 1)
            vector.wait_ge(vsem, 2)  # s2 retired
            nc.vector.reduce_sum(r[:, 1:2], s2[:], axis=X).then_inc(vsem, 1)
            vector.wait_ge(vsem, 3)  # ts2 retired
            nc.vector.reduce_sum(r[:, 4:5], ts2[:], axis=X).then_inc(vsem, 1)

    nc.compile()
    return nc


def _get_nc():
    if "nc" not in _CACHE:
        _CACHE["nc"] = _build()
    return _CACHE["nc"]


def make_in_maps(logits: np.ndarray, targets: np.ndarray) -> list[dict]:
    logits = np.ascontiguousarray(logits, dtype=np.float32)
    t32 = np.asarray(targets).astype(np.float32)  # values are 0/1; lossless
    in_maps = []
    for k in range(NCORES):
        sl = slice(k * SHARD, (k + 1) * SHARD)
        xk = np.empty((P, 2 * F), np.float32)
        xk[:, 0:F] = logits[sl].reshape(P, F)
        xk[:, F : 2 * F] = t32[sl].reshape(P, F)
        in_maps.append({"x": xk})
    return in_maps


def combine(outs: np.ndarray) -> np.ndarray:
    """All-reduce the [NCORES, P, 5] partials and apply the closed form."""
    tot = outs.astype(np.float64).sum(axis=(0, 1))
    g1, g2, c, m1, m2 = tot
    n_pos = c
    n_neg = float(N) - c
    sp1 = c - m1
    sp2 = c - 2.0 * m1 + m2
    sn1 = g1 - m1
    sn2 = g2 - m2
    loss = (n_neg * sp2 + 2.0 * sp1 * sn1 + n_pos * sn2) / (n_pos * n_neg)
    return np.array(loss, dtype=np.float32)


def kernel(logits: np.ndarray, targets: np.ndarray, **run_kwargs):
    nc = _get_nc()
    res = bass_utils.run_bass_kernel_spmd(
        nc, make_in_maps(logits, targets), core_ids=list(range(NCORES)), **run_kwargs
    )
    outs = np.stack([r["o"] for r in res.results])  # [8, 128, 5]
    out = combine(outs)
    _CACHE["last_results"] = res
    return out
